# revision 44
# baseline (speedup 1.0000x reference)
"""Trainium2 Bass kernel for nn_MMN_7361573945989 (MatchNet corr/attention).

Math (per batch b):
  qn_l = l2norm_c(fq_l); sn_l = l2norm_c(fs_l)           l in {4, 3}
  logits[p, q] = TEMP * (w0 * qn4.T@sn4 + w1 * qn3.T@sn3)[p, q]
  attn = softmax_q(logits)
  att_fq[c, p] = sum_q attn[p, q] * f_s[c, q]
  fq_out = l2norm_c(f_q) + l2norm_c(att_fq) * ATT_WT
  returns (fq_out, att_fq)

Sharding: 8 cores = 2 batches x 4 query-pixel shards of 900.

Per-core kernel (transposed orientation, logits live as [q, p] tiles):
  - host pre-tiles all feature tensors into DMA-friendly bf16 layouts and
    zero-pads the support dim 3600 -> 3712 = 29*128 (no ragged chunks)
  - query feats are normalized and scaled by TEMP*w_l on device in prep
    (3-phase: squares split ACT/DVE -> e0-weight partition-sum matmuls ->
    Newton rsqrt from the constant seed C^-1/2 -> streaming bf16 scaling
    muls emitted in ci order so the qc=0 logits start immediately)
  - support feats stream RAW: layer4/layer3 dot products accumulate in
    separate PSUM groups; the per-support-pixel 1/||s_q|| scales apply
    post-matmul as per-partition [q,1] scalars in two DVE ops
  - support sum-of-squares: DVE squares + contiguous pairwise ci-folds,
    then a tiny part.T @ ones matmul yields [q,1] norm columns; Newton
    rsqrt (mult/add only, no ACT tables, no InstReciprocal)
  - every lhsT is loaded once per p-block pair (ldweights=False on the
    second matmul); partition sums use an e0 basis-column weight so the
    output is M=128 (M=1 matmuls are ~35% slower)
  - softmax without max-subtraction: logits = 20*(w.cos) are bounded
  - denominators + Y = exp @ f_s.T run in phase B ([c, p] orientation);
    att = Y/denom; the att_fq l2norm uses ||Y|| so the denominator
    cancels; 1/x and rsqrt on the ACT table (fine at this tolerance)
  - first support chunks live in a pool allocated before the prep pools
    (pool aliasing otherwise stalls their DMAs behind prep\'s last reads)
"""

import sys
from contextlib import ExitStack

import numpy as np

sys.path.insert(0, "/opt/trn_rl_repo")

import concourse.bass as bass  # noqa: E402
import concourse.tile as tile  # noqa: E402
from concourse import mybir  # noqa: E402
from concourse.bass_utils import run_bass_kernel_spmd  # noqa: E402

B, H, W = 2, 60, 60
HW = H * W  # 3600
HWP = 3712  # padded to 29*128
C3, C4, CV = 1024, 2048, 512
TEMP = 20.0
ATT_WT = 0.3
NCORES = 8
PSH = 4  # query-pixel shards per batch
P = HW // PSH  # 900 query pixels per core
PB = P // 2  # 450, p-block (one PSUM bank of fp32)
NQC = HWP // 128  # 29 support-pixel chunks
QT = HW - (NQC - 1) * 128  # 16 real rows in the tail chunk
NC4, NC3, NCV = C4 // 128, C3 // 128, CV // 128  # 16, 8, 4
NCI = NC4 + NC3  # 24 combined channel chunks

F32 = mybir.dt.float32
BF16 = mybir.dt.bfloat16
NP_BF16 = mybir.dt.np(BF16)
AF = mybir.ActivationFunctionType
MUL = mybir.AluOpType.mult
ADD = mybir.AluOpType.add

_MAX_WAITS_PER_INST = 1


def _patched_drain_and_barrier(self, tick_clock, wait_clock):
    """Tile's kernel-tail drain carries one sem wait per engine/queue; the
    walrus build used here accepts only one sync wait per CTRL instruction.
    Split the waits across extra sync-engine nops."""
    drain_inst = self.nc.sync.drain()
    wait_clock.add_sem_waits(
        drain_inst.ins, tile.ScopedClock({None: tick_clock.global_clock})
    )
    si = drain_inst.ins.sync_info
    if si is not None and len(si.on_wait) > _MAX_WAITS_PER_INST:
        waits = list(si.on_wait)
        drain_inst.ins.sync_info = mybir.SyncInfo(
            on_wait=waits[:_MAX_WAITS_PER_INST], on_update=list(si.on_update)
        )
        for i in range(_MAX_WAITS_PER_INST, len(waits), _MAX_WAITS_PER_INST):
            nop = self.nc.sync.nop()
            nop.ins.sync_info = mybir.SyncInfo(
                on_wait=waits[i : i + _MAX_WAITS_PER_INST], on_update=[]
            )
    self.nc.all_engine_barrier()
    assert self.sems is not None
    popped = self.nc._tile_sem_poison_stack.pop()
    assert popped is self._sem_poison
    self.nc.clear_and_free_semaphores(list(self.sems.allocated().values()))
    self.nc.all_engine_barrier()


tile.TileContext._drain_and_barrier = _patched_drain_and_barrier


def _split_sync_waits(nc, max_waits=_MAX_WAITS_PER_INST):
    """Walrus here accepts at most one sync wait per instruction; move excess
    waits onto same-engine nops inserted immediately before the instruction."""
    ctr = 0
    for f in nc.m.functions:
        for blk in f.blocks:
            insts = list(blk.instructions)
            out = []
            changed = False
            for inst in insts:
                si = inst.sync_info
                if si is not None and len(si.on_wait) > max_waits:
                    waits = list(si.on_wait)
                    for i0 in range(max_waits, len(waits), max_waits):
                        ctr += 1
                        nop = mybir.InstNoOp(
                            name=f"waitsplit-{ctr}",
                            engine=inst.engine,
                            bass_nofuse=True,
                            sync_info=mybir.SyncInfo(
                                on_wait=waits[i0 : i0 + max_waits], on_update=[]
                            ),
                        )
                        nc.register_instruction(nop, overwrite=True)
                        out.append(nop)
                    inst.sync_info = mybir.SyncInfo(
                        on_wait=waits[:max_waits], on_update=list(si.on_update)
                    )
                    changed = True
                out.append(inst)
            if changed:
                blk.instructions = out


def build():
    nc = bass.Bass()
    # host layouts (see make_in_maps):
    #   s4t[qc*128 + c, ci*128 + q] = fs_l4[b, ci*128 + c, qc*128 + q]
    #   q4t[c, ci*900 + p]          = fq_l4[b, ci*128 + c, shard p]
    #   vtd[q, v]                   = f_s[b, v, q] (padded rows zero)
    #   fqd[c, ci*900 + p]          = f_q[b, ci*128 + c, shard p]
    s4t = nc.dram_tensor("s4t", [HWP, C4], BF16, kind="ExternalInput")
    s3t = nc.dram_tensor("s3t", [HWP, C3], BF16, kind="ExternalInput")
    q4t = nc.dram_tensor("q4t", [128, NC4 * P], BF16, kind="ExternalInput")
    q3t = nc.dram_tensor("q3t", [128, NC3 * P], BF16, kind="ExternalInput")
    vtd = nc.dram_tensor("vtd", [HWP, CV], BF16, kind="ExternalInput")
    fqd = nc.dram_tensor("fqd", [128, NCV * P], F32, kind="ExternalInput")
    wv = nc.dram_tensor("wv", [1, 2], F32, kind="ExternalInput")  # [T*w0, T*w1]
    att_o = nc.dram_tensor("att_o", [CV, P], F32, kind="ExternalOutput")
    fq_o = nc.dram_tensor("fq_o", [CV, P], F32, kind="ExternalOutput")

    with tile.TileContext(nc) as tc:
        with ExitStack() as octx:
            cpool = octx.enter_context(tc.tile_pool(name="const", bufs=1))
            ones_col = cpool.tile([128, 1], BF16)
            nc.gpsimd.memset(ones_col[:], 1.0)
            ones_row = cpool.tile([1, 128], F32)
            nc.gpsimd.memset(ones_row[:], 1.0)
            w_sb = cpool.tile([1, 2], F32)
            nc.sync.dma_start(w_sb[:], wv[:])
            w_col = cpool.tile([128, 2], F32)
            ones_row_bf = cpool.tile([1, 128], BF16)
            nc.gpsimd.memset(ones_row_bf[:], 1.0)
            # e0[k, m] = (m == 0): lhsT for partition sums with a full
            # M=128 output (row 0 = sum); M=1 matmuls run ~35% slower
            e0 = cpool.tile([128, 128], BF16)
            nc.gpsimd.memset(e0[:], 0.0)
            nc.gpsimd.memset(e0[:, 0:1], 1.0)
            zeros_pb = cpool.tile([128, PB], F32)
            nc.gpsimd.memset(zeros_pb[:], 0.0)

            def act_table(out, in_, func, scale=1.0):
                # raw InstActivation emit: Reciprocal/Rsqrt are blocked by
                # the bass wrapper for accuracy reasons; the ~1e-3 table
                # error is fine at this kernel's tolerance
                eng = nc.scalar
                ins = [eng.lower_ap(in_)]
                for v in (0.0, float(scale), 0.0):  # bias, scale, alpha
                    ins.append(
                        mybir.ImmediateValue(dtype=mybir.dt.float32, value=v)
                    )
                return eng.add_instruction(
                    mybir.InstActivation(
                        name=nc.get_next_instruction_name(),
                        func=func,
                        ins=ins,
                        outs=[eng.lower_ap(out)],
                    )
                )

            pers = octx.enter_context(tc.tile_pool(name="pers", bufs=1))
            qns = pers.tile([128, NCI * P], BF16)  # scaled query feats (ci, p)
            fqn = pers.tile([128, NCV * P], F32)  # normalized f_q (ci, p)
            expT = pers.tile([128, NQC * P], BF16)  # exp(logits) (qc; q, p)
            # first support chunks in their own pool (allocated before the
            # prep pools) so their DMAs are not ordered behind prep's last
            # reads; released before phase B to return the SBUF
            NWARM = 3
            warm_ctx = ExitStack()
            warmpool = warm_ctx.enter_context(tc.tile_pool(name="warm", bufs=1))
            # nw factors outlive the prep pools: the 48 streaming scale-muls
            # read them, and keeping them out of the prep pools lets prep
            # release early (unblocking chunk DMAs + logits PSUM)
            nwpool = warm_ctx.enter_context(tc.tile_pool(name="nwp", bufs=1))
            warm_sc = warmpool.tile([128, NWARM * NCI * 128], BF16)

            def load_warm():
                wv4 = warm_sc[:].rearrange("c (wq x) -> c wq x", wq=NWARM)
                s4v = s4t[0 : NWARM * 128, :].rearrange(
                    "(wq c) x -> c wq x", c=128
                )
                s3v = s3t[0 : NWARM * 128, :].rearrange(
                    "(wq c) x -> c wq x", c=128
                )
                nc.sync.dma_start(wv4[:, :, 0 : NC4 * 128], s4v)
                nc.sync.dma_start(wv4[:, :, NC4 * 128 : NCI * 128], s3v)
            # zero the tail-chunk region; exp writes only rows [0:QT] there
            nc.gpsimd.memset(expT[:, (NQC - 1) * P : NQC * P], 0.0)

            # broadcast T*w across partitions once: [1,2] -> [128,2]
            with tc.tile_pool(name="wps", bufs=1, space="PSUM") as wps:
                # dummy matmul burst while the PE waits on the first DMAs:
                # ~4us of activity flips the HAM clock-gate to 2.4 GHz so
                # the prep matmuls don't run at the cold 1.2 GHz rate
                warm_ps = wps.tile([128, 128], F32, tag="warmup")
                for i in range(28):
                    mm = nc.tensor.matmul(
                        warm_ps[:], e0[:], e0[:],
                        start=(i == 0), stop=(i == 27),
                        skip_group_check=True,
                    )
                    if i > 0:
                        mm.ins.ldweights = False
                w_ps = wps.tile([128, 2], F32)
                nc.tensor.matmul(w_ps[:], ones_row[:], w_sb[:])
                nc.scalar.copy(w_col[:], w_ps[:])

            # ---------------- prep: query-side normalization ----------------
            # 3 phases so the PE's prep work is not serialized behind the
            # DVE scaling cascade: (1) squares + sum-of-square matmul rows,
            # (2) broadcast + Newton rsqrt, (3) in-place scaling.
            for a, b in ((0, 2), (2, 8), (8, NC4)):
                nc.sync.dma_start(
                    qns[:, a * P : b * P], q4t[:, a * P : b * P]
                )
            for a, b in ((0, 2), (2, NC3)):
                nc.sync.dma_start(
                    qns[:, (NC4 + a) * P : (NC4 + b) * P],
                    q3t[:, a * P : b * P],
                )
            nc.sync.dma_start(fqn[:], fqd[:])
            with ExitStack() as pctx:
                sqpool = pctx.enter_context(tc.tile_pool(name="prepsq", bufs=2))
                mini = pctx.enter_context(tc.tile_pool(name="prepmini", bufs=2))
                pps = pctx.enter_context(
                    tc.tile_pool(name="prepps", bufs=1, space="PSUM")
                )

                qlayers = [(0, NC4, 0), (NC4, NC3, 1)]
                nw_all = {}
                # per-layer pipeline: squares (alternating ACT/DVE) ->
                # sum-of-squares matmuls -> broadcast -> Newton rsqrt ->
                # bf16 w*ninv factor -> in-place scaling.  Layer-4 finishes
                # first so the qc=0 logits matmuls can start while layer-3
                # is still being normalized.
                for ci0, n_ci, wi in qlayers:
                    y0 = float((n_ci * 128) ** -0.5)
                    ss = [
                        pps.tile(
                            [128, PB], F32, tag=f"ss{wi}{pb}", name=f"ss{wi}{pb}"
                        )
                        for pb in range(2)
                    ]
                    groups = [2, 2] + [4] * ((n_ci - 4) // 4)
                    g0 = 0
                    for gi, g in enumerate(groups):
                        sqq = sqpool.tile([128, 4 * P], BF16, tag="sqq")
                        # split each group's squares across ACT and DVE so
                        # the sum-of-squares matmuls are never starved
                        h = g // 2
                        lo = (ci0 + g0) * P
                        nc.scalar.square(
                            sqq[:, 0 : h * P], qns[:, lo : lo + h * P]
                        )
                        nc.vector.tensor_mul(
                            sqq[:, h * P : g * P],
                            qns[:, lo + h * P : lo + g * P],
                            qns[:, lo + h * P : lo + g * P],
                        )
                        for k in range(g):
                            ci = g0 + k
                            for pb in range(2):
                                mm = nc.tensor.matmul(
                                    ss[pb][:],
                                    e0[:],
                                    sqq[:, k * P + pb * PB : k * P + pb * PB + PB],
                                    start=(ci == 0),
                                    stop=(ci == n_ci - 1),
                                )
                                if ci > 0 or pb > 0:
                                    mm.ins.ldweights = False
                        g0 += g
                    nws = []
                    for pb in range(2):
                        u = mini.tile([1, PB], BF16, tag="u")
                        nc.scalar.copy(u[:], ss[pb][0:1, :])
                        bc = pps.tile([128, PB], F32, tag="bc", name=f"bc{wi}{pb}")
                        nc.tensor.matmul(bc[:], ones_row_bf[:], u[:])
                        y1 = mini.tile([128, PB], F32, tag="y1")
                        nc.vector.tensor_scalar(
                            out=y1[:],
                            in0=bc[:],
                            scalar1=-0.5 * y0 * y0 * y0,
                            scalar2=1.5 * y0,
                            op0=MUL,
                            op1=ADD,
                        )
                        t = mini.tile([128, PB], F32, tag="t")
                        nc.vector.tensor_mul(t[:], y1[:], y1[:])
                        nc.vector.tensor_mul(t[:], t[:], bc[:])
                        nc.vector.tensor_scalar(
                            out=t[:], in0=t[:], scalar1=-0.5, scalar2=1.5,
                            op0=MUL, op1=ADD,
                        )
                        ninv = mini.tile([128, PB], F32, tag="ninv")
                        nc.vector.tensor_mul(ninv[:], t[:], y1[:])
                        if pb == 0:
                            nw = nwpool.tile([128, P], BF16, tag=f"nw{wi}")
                            nws.append(nw)
                        nc.vector.scalar_tensor_tensor(
                            nws[0][:, pb * PB : (pb + 1) * PB],
                            ninv[:], w_col[:, wi : wi + 1], zeros_pb[:],
                            MUL, ADD,
                        )
                    nw_all[wi] = nws[0]
                    if wi == 0:
                        # warm-chunk DMAs emitted here: late enough that the
                        # first squares' DMA waits don't cover them, early
                        # enough to be resident before the logits start
                        load_warm()
                # the in-place scaling muls run LAST, in ci order, so the
                # layer-3 norm chain is not stuck behind them on the DVE
                for ci in range(NCI):
                    wi = 0 if ci < NC4 else 1
                    sl = slice(ci * P, (ci + 1) * P)
                    nc.vector.tensor_mul(qns[:, sl], qns[:, sl], nw_all[wi][:])

            # ------------- main: stream support chunks, logits, exp -------------
            with ExitStack() as mctx:
                scpool = mctx.enter_context(tc.tile_pool(name="sc", bufs=3))
                sqpool = mctx.enter_context(tc.tile_pool(name="msq", bufs=3))
                fscpool = mctx.enter_context(tc.tile_pool(name="mfsc", bufs=2))
                invpool = mctx.enter_context(tc.tile_pool(name="minv", bufs=2))
                cmbpool = mctx.enter_context(tc.tile_pool(name="mcmb", bufs=3))
                lps = mctx.enter_context(
                    tc.tile_pool(name="logits", bufs=1, space="PSUM")
                )
                nps = mctx.enter_context(
                    tc.tile_pool(name="normps", bufs=2, space="PSUM")
                )

                for qc in range(NQC):
                    qn = 128 if qc < NQC - 1 else QT
                    r0 = qc * 128
                    if qc < NWARM:
                        sc = warm_sc[:, qc * NCI * 128 : (qc + 1) * NCI * 128]
                    else:
                        sc = scpool.tile([128, NCI * 128], BF16, tag="sc")
                        nc.sync.dma_start(
                            sc[:, 0 : NC4 * 128], s4t[r0 : r0 + 128, :]
                        )
                        nc.sync.dma_start(
                            sc[:, NC4 * 128 : NCI * 128], s3t[r0 : r0 + 128, :]
                        )

                    # support sum-of-squares -> [q,1] norm columns via
                    # pairwise ci folds (squares + first folds on GpSimd)
                    sq = sqpool.tile([128, NCI * 128], BF16, tag="sq")
                    if qc < 2:
                        # DVE is busy with the prep scaling muls here
                        nc.scalar.square(sq[:], sc[:])
                    else:
                        nc.vector.tensor_mul(sq[:], sc[:], sc[:])
                    fsc = fscpool.tile([128, 1536], F32, tag="fsc")
                    nc.vector.tensor_add(
                        fsc[:, 0:1024], sq[:, 0:1024], sq[:, 1024:2048]
                    )
                    nc.vector.tensor_add(
                        fsc[:, 1024:1536], sq[:, 2048:2560], sq[:, 2560:3072]
                    )
                    for lo, n in ((0, 512), (0, 256), (0, 128), (1024, 256), (1024, 128)):
                        nc.vector.tensor_add(
                            fsc[:, lo : lo + n],
                            fsc[:, lo : lo + n],
                            fsc[:, lo + n : lo + 2 * n],
                        )
                    ncps = nps.tile([128, 2], F32, tag="ncp", name=f"ncp{qc}")

                    # logits: D4/D3 accumulate separately; lhsT is the raw
                    # support chunk, reused across both p-blocks
                    psD = [
                        [
                            lps.tile(
                                [128, PB], F32, tag=f"D{ln}{pb}",
                                name=f"D{ln}{pb}", bufs=(2 if ln == 0 else 1),
                            )
                            for pb in range(2)
                        ]
                        for ln in range(2)
                    ]
                    for ci in range(NCI):
                        ln = 0 if ci < NC4 else 1
                        lhsT = sc[:, ci * 128 : (ci + 1) * 128]
                        for pb in range(2):
                            mm = nc.tensor.matmul(
                                psD[ln][pb][:],
                                lhsT,
                                qns[:, ci * P + pb * PB : ci * P + pb * PB + PB],
                                start=(ci == 0 or ci == NC4),
                                stop=(ci == NC4 - 1 or ci == NCI - 1),
                            )
                            if pb == 1:
                                # second matmul of the pair reuses the
                                # weights loaded by the first
                                mm.ins.ldweights = False
                        if ci == 12:
                            # norm-column matmuls tucked behind the first
                            # logits pairs so they never gate the PE; bf16
                            # casts avoid the fp32 double-pumped matmul
                            fbf = invpool.tile([128, 256], BF16, tag="fbf")
                            nc.scalar.copy(fbf[:, 0:128], fsc[:, 0:128])
                            nc.scalar.copy(fbf[:, 128:256], fsc[:, 1024:1152])
                            for ln2 in range(2):
                                nc.tensor.matmul(
                                    ncps[:, ln2 : ln2 + 1],
                                    fbf[:, ln2 * 128 : (ln2 + 1) * 128],
                                    ones_col[:],
                                    skip_group_check=True,
                                )
                    # inv = (sum sq)^(-1/2): Newton rsqrt from constant seeds
                    inv1 = invpool.tile([128, 2], F32, tag="i1")
                    for ln, c in ((0, C4), (1, C3)):
                        y0 = float(c**-0.5)
                        nc.vector.tensor_scalar(
                            out=inv1[:, ln : ln + 1],
                            in0=ncps[:, ln : ln + 1],
                            scalar1=-0.5 * y0 * y0 * y0,
                            scalar2=1.5 * y0,
                            op0=MUL,
                            op1=ADD,
                        )
                    t2 = invpool.tile([128, 2], F32, tag="t2")
                    nc.vector.tensor_mul(t2[:], inv1[:], inv1[:])
                    nc.vector.tensor_mul(t2[:], t2[:], ncps[:])
                    nc.vector.tensor_scalar(
                        out=t2[:], in0=t2[:], scalar1=-0.5, scalar2=1.5,
                        op0=MUL, op1=ADD,
                    )
                    inv = invpool.tile([128, 2], F32, tag="inv")
                    nc.vector.tensor_mul(inv[:], t2[:], inv1[:])
                    for pb in range(2):
                        tmp = cmbpool.tile([128, PB], F32, tag="tmp")
                        nc.vector.scalar_tensor_tensor(
                            tmp[:], psD[1][pb][:], inv[:, 1:2], zeros_pb[:],
                            MUL, ADD,
                        )
                        cmb = cmbpool.tile([128, PB], F32, tag="cmb")
                        nc.vector.scalar_tensor_tensor(
                            cmb[:], psD[0][pb][:], inv[:, 0:1], tmp[:], MUL, ADD
                        )
                        esl = expT[
                            0:qn, qc * P + pb * PB : qc * P + pb * PB + PB
                        ]
                        nc.scalar.activation(esl, cmb[0:qn, :], AF.Exp)

            warm_ctx.close()

            # ---------------- phase B: attention-weighted values ----------------
            with ExitStack() as bctx:
                vpool = bctx.enter_context(tc.tile_pool(name="vtp", bufs=1))
                bps = bctx.enter_context(
                    tc.tile_pool(name="bps", bufs=1, space="PSUM")
                )
                bsq = bctx.enter_context(tc.tile_pool(name="bsq", bufs=2))
                bmini = bctx.enter_context(tc.tile_pool(name="bmini", bufs=1))
                batt = bctx.enter_context(tc.tile_pool(name="batt", bufs=1))
                bout = bctx.enter_context(tc.tile_pool(name="bout", bufs=2))

                # f_q channel-normalization (moved here: only needed by
                # the epilogue; hides under the Y matmuls)
                fsq = bsq.tile([128, NCV * P], BF16, tag="fsq", bufs=1)
                nc.vector.tensor_mul(fsq[:], fqn[:], fqn[:])
                fss = [
                    bps.tile([128, PB], F32, tag=f"ssy{pb}", name=f"fss{pb}")
                    for pb in range(2)
                ]
                for ci in range(NCV):
                    for pb in range(2):
                        mm = nc.tensor.matmul(
                            fss[pb][:],
                            e0[:],
                            fsq[:, ci * P + pb * PB : ci * P + pb * PB + PB],
                            start=(ci == 0),
                            stop=(ci == NCV - 1),
                        )
                        if ci > 0 or pb > 0:
                            mm.ins.ldweights = False
                y0f = float(CV**-0.5)
                for pb in range(2):
                    u = bmini.tile([1, PB], BF16, tag=f"uf{pb}")
                    nc.scalar.copy(u[:], fss[pb][0:1, :])
                    bc = bps.tile([128, PB], F32, tag="bcscr", name=f"fbc{pb}")
                    nc.tensor.matmul(bc[:], ones_row_bf[:], u[:])
                    y1 = bmini.tile([128, PB], F32, tag="y1f")
                    nc.vector.tensor_scalar(
                        out=y1[:], in0=bc[:], scalar1=-0.5 * y0f * y0f * y0f,
                        scalar2=1.5 * y0f, op0=MUL, op1=ADD,
                    )
                    t = bmini.tile([128, PB], F32, tag="tf")
                    nc.vector.tensor_mul(t[:], y1[:], y1[:])
                    nc.vector.tensor_mul(t[:], t[:], bc[:])
                    nc.vector.tensor_scalar(
                        out=t[:], in0=t[:], scalar1=-0.5, scalar2=1.5,
                        op0=MUL, op1=ADD,
                    )
                    ninv = bmini.tile([128, PB], F32, tag=f"ninvf{pb}")
                    nc.vector.tensor_mul(ninv[:], t[:], y1[:])
                    for ci in range(NCV):
                        sl = slice(ci * P + pb * PB, ci * P + pb * PB + PB)
                        nc.vector.tensor_mul(fqn[:, sl], fqn[:, sl], ninv[:])

                # stream f_s.T directly as bf16 (pad rows are zero on host)
                vt_all = vpool.tile([128, NQC * CV], BF16)
                vtv = vt_all[:].rearrange("q (qc v) -> q qc v", qc=NQC)
                srcv = vtd[:].rearrange("(qc q) v -> q qc v", q=128)
                for qc0 in range(0, NQC, 8):
                    g = min(8, NQC - qc0)
                    nc.sync.dma_start(
                        vtv[:, qc0 : qc0 + g, :], srcv[:, qc0 : qc0 + g, :]
                    )

                # softmax denominators + 1/denominator broadcast; the psum
                # pool is scoped so its banks free up for the Y matmuls
                bcd_sb, bcd_raw = [], []
                with tc.tile_pool(name="dnps", bufs=1, space="PSUM") as dnps:
                    dns = [
                        dnps.tile(
                            [128, PB], F32, tag=f"dn{pb}", name=f"dn{pb}"
                        )
                        for pb in range(2)
                    ]
                    for qc in range(NQC):
                        for pb in range(2):
                            mm = nc.tensor.matmul(
                                dns[pb][:],
                                e0[:],
                                expT[:, qc * P + pb * PB : qc * P + pb * PB + PB],
                                start=(qc == 0),
                                stop=(qc == NQC - 1),
                            )
                            if qc > 0 or pb > 0:
                                mm.ins.ldweights = False
                    for pb in range(2):
                        u = bmini.tile([1, PB], F32, tag=f"ud{pb}")
                        nc.scalar.copy(u[:], dns[pb][0:1, :])
                        bcp = bps.tile([128, PB], F32, tag="bcscr", name=f"bd{pb}")
                        nc.tensor.matmul(bcp[:], ones_row[:], u[:])
                        raw = bmini.tile([128, PB], F32, tag=f"dnraw{pb}")
                        nc.scalar.copy(raw[:], bcp[:])
                        inv = bmini.tile([128, PB], F32, tag=f"dninv{pb}")
                        act_table(inv[:], bcp[:], AF.Reciprocal)
                        bcd_sb.append(inv)
                        bcd_raw.append(raw)

                yps = bctx.enter_context(
                    tc.tile_pool(name="yps", bufs=2, space="PSUM")
                )
                ssy = [
                    bps.tile([128, PB], F32, tag=f"ssy{pb}", name=f"ssy{pb}")
                    for pb in range(2)
                ]
                att_sb = {}
                for cb in range(NCV):
                    ys = [
                        yps.tile([128, PB], F32, tag=f"y{pb}", name=f"y{cb}_{pb}")
                        for pb in range(2)
                    ]
                    for qc in range(NQC):
                        lhsT = vt_all[:, qc * CV + cb * 128 : qc * CV + (cb + 1) * 128]
                        for pb in range(2):
                            mm = nc.tensor.matmul(
                                ys[pb][:],
                                lhsT,
                                expT[:, qc * P + pb * PB : qc * P + pb * PB + PB],
                                start=(qc == 0),
                                stop=(qc == NQC - 1),
                            )
                            if pb == 1:
                                mm.ins.ldweights = False
                    for pb in range(2):
                        att = batt.tile(
                            [128, PB], F32, tag=f"att{cb}_{pb}", name=f"att{cb}_{pb}"
                        )
                        nc.vector.tensor_mul(att[:], ys[pb][:], bcd_sb[pb][:])
                        att_sb[(cb, pb)] = att
                        nc.sync.dma_start(
                            att_o[cb * 128 : (cb + 1) * 128, pb * PB : (pb + 1) * PB],
                            att[:],
                        )
                        sqy = bsq.tile([128, PB], BF16, tag="sqy")
                        nc.scalar.square(sqy[:], ys[pb][:])
                        mm = nc.tensor.matmul(
                            ssy[pb][:],
                            e0[:],
                            sqy[:],
                            start=(cb == 0),
                            stop=(cb == NCV - 1),
                        )
                        if pb == 1:
                            mm.ins.ldweights = False

                for pb in range(2):
                    u = bmini.tile([1, PB], F32, tag=f"us{pb}")
                    nc.scalar.copy(u[:], ssy[pb][0:1, :])
                    bcp = bps.tile([128, PB], F32, tag="bcscr", name=f"bs{pb}")
                    nc.tensor.matmul(bcp[:], ones_row[:], u[:])
                    # rsqrt(ssy/ATT_WT^2) = 0.3/||Y|| in one ACT op
                    sinv = bmini.tile([128, PB], F32, tag=f"sinv{pb}")
                    act_table(
                        sinv[:], bcp[:], AF.Rsqrt,
                        scale=float(1.0 / (ATT_WT * ATT_WT)),
                    )
                    # fq = fqn + att * (denom * 0.3/||Y||)
                    s2 = bmini.tile([128, PB], F32, tag=f"s2{pb}")
                    nc.vector.tensor_mul(s2[:], bcd_raw[pb][:], sinv[:])
                    for cb in range(NCV):
                        t = bout.tile([128, PB], F32, tag=f"t{pb}")
                        nc.vector.tensor_mul(t[:], att_sb[(cb, pb)][:], s2[:])
                        f_sb = bout.tile([128, PB], F32, tag=f"f{pb}")
                        nc.vector.tensor_add(
                            f_sb[:],
                            t[:],
                            fqn[:, cb * P + pb * PB : cb * P + pb * PB + PB],
                        )
                        nc.sync.dma_start(
                            fq_o[cb * 128 : (cb + 1) * 128, pb * PB : (pb + 1) * PB],
                            f_sb[:],
                        )
    _split_sync_waits(nc)
    return nc


def _tile_support(x, n_ci):
    """[C, HW] f32 -> [HWP, C] bf16 with s[qc*128+c, ci*128+q] layout."""
    a = np.asarray(x, np.float32).reshape(n_ci, 128, HW)
    a = np.concatenate(
        [a, np.zeros((n_ci, 128, HWP - HW), np.float32)], axis=2
    )
    a = a.reshape(n_ci, 128, NQC, 128).transpose(2, 1, 0, 3).reshape(HWP, n_ci * 128)
    return np.ascontiguousarray(a.astype(NP_BF16))


def _tile_query(x, n_ci, dtype):
    """[C, P] -> [128, n_ci*P] with q[c, ci*P + p] layout."""
    a = np.asarray(x, np.float32).reshape(n_ci, 128, P).transpose(1, 0, 2)
    return np.ascontiguousarray(a.reshape(128, n_ci * P).astype(dtype))


def make_in_maps(fq_l3, fs_l3, fq_l4, fs_l4, f_q, f_s, w_red):
    wvec = np.asarray(
        [[TEMP * float(w_red[0]), TEMP * float(w_red[1])]], dtype=np.float32
    )
    per_batch = []
    for b in range(B):
        s4 = _tile_support(np.asarray(fs_l4, np.float32)[b].reshape(C4, HW), NC4)
        s3 = _tile_support(np.asarray(fs_l3, np.float32)[b].reshape(C3, HW), NC3)
        vt = np.zeros((HWP, CV), np.float32)
        vt[:HW] = np.asarray(f_s, np.float32)[b].reshape(CV, HW).T
        vt = np.ascontiguousarray(vt.astype(NP_BF16))
        per_batch.append((s4, s3, vt))
    q4f = np.asarray(fq_l4, np.float32).reshape(B, C4, HW)
    q3f = np.asarray(fq_l3, np.float32).reshape(B, C3, HW)
    fqf = np.asarray(f_q, np.float32).reshape(B, CV, HW)
    in_maps = []
    for k in range(NCORES):
        b, j = divmod(k, PSH)
        sl = slice(j * P, (j + 1) * P)
        s4, s3, vt = per_batch[b]
        in_maps.append(
            {
                "s4t": s4,
                "s3t": s3,
                "vtd": vt,
                "q4t": _tile_query(q4f[b][:, sl], NC4, NP_BF16),
                "q3t": _tile_query(q3f[b][:, sl], NC3, NP_BF16),
                "fqd": _tile_query(fqf[b][:, sl], NCV, np.float32),
                "wv": wvec,
            }
        )
    return in_maps


def gather_outputs(results):
    att = np.empty((B, CV, HW), np.float32)
    fqo = np.empty((B, CV, HW), np.float32)
    for k in range(NCORES):
        b, j = divmod(k, PSH)
        sl = slice(j * P, (j + 1) * P)
        att[b][:, sl] = results[k]["att_o"]
        fqo[b][:, sl] = results[k]["fq_o"]
    return (
        fqo.reshape(B, CV, H, W),
        att.reshape(B, CV, H, W),
    )


def kernel(fq_l3, fs_l3, fq_l4, fs_l4, f_q, f_s, w_red, trace=False):
    nc = build()
    in_maps = make_in_maps(fq_l3, fs_l3, fq_l4, fs_l4, f_q, f_s, w_red)
    res = run_bass_kernel_spmd(nc, in_maps, core_ids=list(range(NCORES)), trace=trace)
    out = gather_outputs(res.results)
    if trace:
        return out, res
    return out


# revision 46
# speedup vs baseline: 1.0042x; 1.0042x over previous
"""Trainium2 Bass kernel for nn_MMN_7361573945989 (MatchNet corr/attention).

Math (per batch b):
  qn_l = l2norm_c(fq_l); sn_l = l2norm_c(fs_l)           l in {4, 3}
  logits[p, q] = TEMP * (w0 * qn4.T@sn4 + w1 * qn3.T@sn3)[p, q]
  attn = softmax_q(logits)
  att_fq[c, p] = sum_q attn[p, q] * f_s[c, q]
  fq_out = l2norm_c(f_q) + l2norm_c(att_fq) * ATT_WT
  returns (fq_out, att_fq)

Sharding: 8 cores = 2 batches x 4 query-pixel shards of 900.

Per-core kernel (transposed orientation, logits live as [q, p] tiles):
  - host pre-tiles all feature tensors into DMA-friendly bf16 layouts and
    zero-pads the support dim 3600 -> 3712 = 29*128 (no ragged chunks)
  - query feats are normalized and scaled by TEMP*w_l on device in prep
    (3-phase: squares split ACT/DVE -> e0-weight partition-sum matmuls ->
    Newton rsqrt from the constant seed C^-1/2 -> streaming bf16 scaling
    muls emitted in ci order so the qc=0 logits start immediately)
  - support feats stream RAW: layer4/layer3 dot products accumulate in
    separate PSUM groups; the per-support-pixel 1/||s_q|| scales apply
    post-matmul as per-partition [q,1] scalars in two DVE ops
  - support sum-of-squares: DVE squares + contiguous pairwise ci-folds,
    then a tiny part.T @ ones matmul yields [q,1] norm columns; Newton
    rsqrt (mult/add only, no ACT tables, no InstReciprocal)
  - every lhsT is loaded once per p-block pair (ldweights=False on the
    second matmul); partition sums use an e0 basis-column weight so the
    output is M=128 (M=1 matmuls are ~35% slower)
  - softmax without max-subtraction: logits = 20*(w.cos) are bounded
  - denominators + Y = exp @ f_s.T run in phase B ([c, p] orientation);
    att = Y/denom; the att_fq l2norm uses ||Y|| so the denominator
    cancels; 1/x and rsqrt on the ACT table (fine at this tolerance)
  - first support chunks live in a pool allocated before the prep pools
    (pool aliasing otherwise stalls their DMAs behind prep\'s last reads)
"""

import sys
from contextlib import ExitStack

import numpy as np

sys.path.insert(0, "/opt/trn_rl_repo")

import concourse.bass as bass  # noqa: E402
import concourse.tile as tile  # noqa: E402
from concourse import mybir  # noqa: E402
from concourse.bass_utils import run_bass_kernel_spmd  # noqa: E402

B, H, W = 2, 60, 60
HW = H * W  # 3600
HWP = 3712  # padded to 29*128
C3, C4, CV = 1024, 2048, 512
TEMP = 20.0
ATT_WT = 0.3
NCORES = 8
PSH = 4  # query-pixel shards per batch
P = HW // PSH  # 900 query pixels per core
PB = P // 2  # 450, p-block (one PSUM bank of fp32)
NQC = HWP // 128  # 29 support-pixel chunks
QT = HW - (NQC - 1) * 128  # 16 real rows in the tail chunk
NC4, NC3, NCV = C4 // 128, C3 // 128, CV // 128  # 16, 8, 4
NCI = NC4 + NC3  # 24 combined channel chunks

F32 = mybir.dt.float32
BF16 = mybir.dt.bfloat16
NP_BF16 = mybir.dt.np(BF16)
AF = mybir.ActivationFunctionType
MUL = mybir.AluOpType.mult
ADD = mybir.AluOpType.add

_MAX_WAITS_PER_INST = 1


def _patched_drain_and_barrier(self, tick_clock, wait_clock):
    """Tile's kernel-tail drain carries one sem wait per engine/queue; the
    walrus build used here accepts only one sync wait per CTRL instruction.
    Split the waits across extra sync-engine nops."""
    drain_inst = self.nc.sync.drain()
    wait_clock.add_sem_waits(
        drain_inst.ins, tile.ScopedClock({None: tick_clock.global_clock})
    )
    si = drain_inst.ins.sync_info
    if si is not None and len(si.on_wait) > _MAX_WAITS_PER_INST:
        waits = list(si.on_wait)
        drain_inst.ins.sync_info = mybir.SyncInfo(
            on_wait=waits[:_MAX_WAITS_PER_INST], on_update=list(si.on_update)
        )
        for i in range(_MAX_WAITS_PER_INST, len(waits), _MAX_WAITS_PER_INST):
            nop = self.nc.sync.nop()
            nop.ins.sync_info = mybir.SyncInfo(
                on_wait=waits[i : i + _MAX_WAITS_PER_INST], on_update=[]
            )
    self.nc.all_engine_barrier()
    assert self.sems is not None
    popped = self.nc._tile_sem_poison_stack.pop()
    assert popped is self._sem_poison
    self.nc.clear_and_free_semaphores(list(self.sems.allocated().values()))
    self.nc.all_engine_barrier()


tile.TileContext._drain_and_barrier = _patched_drain_and_barrier


def _split_sync_waits(nc, max_waits=_MAX_WAITS_PER_INST):
    """Walrus here accepts at most one sync wait per instruction; move excess
    waits onto same-engine nops inserted immediately before the instruction."""
    ctr = 0
    for f in nc.m.functions:
        for blk in f.blocks:
            insts = list(blk.instructions)
            out = []
            changed = False
            for inst in insts:
                si = inst.sync_info
                if si is not None and len(si.on_wait) > max_waits:
                    waits = list(si.on_wait)
                    for i0 in range(max_waits, len(waits), max_waits):
                        ctr += 1
                        nop = mybir.InstNoOp(
                            name=f"waitsplit-{ctr}",
                            engine=inst.engine,
                            bass_nofuse=True,
                            sync_info=mybir.SyncInfo(
                                on_wait=waits[i0 : i0 + max_waits], on_update=[]
                            ),
                        )
                        nc.register_instruction(nop, overwrite=True)
                        out.append(nop)
                    inst.sync_info = mybir.SyncInfo(
                        on_wait=waits[:max_waits], on_update=list(si.on_update)
                    )
                    changed = True
                out.append(inst)
            if changed:
                blk.instructions = out


def build():
    nc = bass.Bass()
    # host layouts (see make_in_maps):
    #   s4t[qc*128 + c, ci*128 + q] = fs_l4[b, ci*128 + c, qc*128 + q]
    #   q4t[c, ci*900 + p]          = fq_l4[b, ci*128 + c, shard p]
    #   vtd[q, v]                   = f_s[b, v, q] (padded rows zero)
    #   fqd[c, ci*900 + p]          = f_q[b, ci*128 + c, shard p]
    s4t = nc.dram_tensor("s4t", [HWP, C4], BF16, kind="ExternalInput")
    s3t = nc.dram_tensor("s3t", [HWP, C3], BF16, kind="ExternalInput")
    q4t = nc.dram_tensor("q4t", [128, NC4 * P], BF16, kind="ExternalInput")
    q3t = nc.dram_tensor("q3t", [128, NC3 * P], BF16, kind="ExternalInput")
    vtd = nc.dram_tensor("vtd", [HWP, CV], BF16, kind="ExternalInput")
    fqd = nc.dram_tensor("fqd", [128, NCV * P], F32, kind="ExternalInput")
    wv = nc.dram_tensor("wv", [1, 2], F32, kind="ExternalInput")  # [T*w0, T*w1]
    att_o = nc.dram_tensor("att_o", [CV, P], F32, kind="ExternalOutput")
    fq_o = nc.dram_tensor("fq_o", [CV, P], F32, kind="ExternalOutput")

    with tile.TileContext(nc) as tc:
        with ExitStack() as octx:
            cpool = octx.enter_context(tc.tile_pool(name="const", bufs=1))
            ones_col = cpool.tile([128, 1], BF16)
            nc.gpsimd.memset(ones_col[:], 1.0)
            ones_row = cpool.tile([1, 128], F32)
            nc.gpsimd.memset(ones_row[:], 1.0)
            w_sb = cpool.tile([1, 2], F32)
            nc.sync.dma_start(w_sb[:], wv[:])
            w_col = cpool.tile([128, 2], F32)
            ones_row_bf = cpool.tile([1, 128], BF16)
            nc.gpsimd.memset(ones_row_bf[:], 1.0)
            # e0[k, m] = (m == 0): lhsT for partition sums with a full
            # M=128 output (row 0 = sum); M=1 matmuls run ~35% slower
            e0 = cpool.tile([128, 128], BF16)
            nc.gpsimd.memset(e0[:], 0.0)
            nc.gpsimd.memset(e0[:, 0:1], 1.0)
            zeros_pb = cpool.tile([128, PB], F32)
            nc.gpsimd.memset(zeros_pb[:], 0.0)

            def act_table(out, in_, func, scale=1.0):
                # raw InstActivation emit: Reciprocal/Rsqrt are blocked by
                # the bass wrapper for accuracy reasons; the ~1e-3 table
                # error is fine at this kernel's tolerance
                eng = nc.scalar
                ins = [eng.lower_ap(in_)]
                for v in (0.0, float(scale), 0.0):  # bias, scale, alpha
                    ins.append(
                        mybir.ImmediateValue(dtype=mybir.dt.float32, value=v)
                    )
                return eng.add_instruction(
                    mybir.InstActivation(
                        name=nc.get_next_instruction_name(),
                        func=func,
                        ins=ins,
                        outs=[eng.lower_ap(out)],
                    )
                )

            pers = octx.enter_context(tc.tile_pool(name="pers", bufs=1))
            qns = pers.tile([128, NCI * P], BF16)  # scaled query feats (ci, p)
            fqn = pers.tile([128, NCV * P], F32)  # normalized f_q (ci, p)
            expT = pers.tile([128, NQC * P], BF16)  # exp(logits) (qc; q, p)
            # first support chunks in their own pool (allocated before the
            # prep pools) so their DMAs are not ordered behind prep's last
            # reads; released before phase B to return the SBUF
            NWARM = 3
            warm_ctx = ExitStack()
            warmpool = warm_ctx.enter_context(tc.tile_pool(name="warm", bufs=1))
            # nw factors outlive the prep pools: the 48 streaming scale-muls
            # read them, and keeping them out of the prep pools lets prep
            # release early (unblocking chunk DMAs + logits PSUM)
            nwpool = warm_ctx.enter_context(tc.tile_pool(name="nwp", bufs=1))
            warm_sc = warmpool.tile([128, NWARM * NCI * 128], BF16)

            def load_warm():
                wv4 = warm_sc[:].rearrange("c (wq x) -> c wq x", wq=NWARM)
                s4v = s4t[0 : NWARM * 128, :].rearrange(
                    "(wq c) x -> c wq x", c=128
                )
                s3v = s3t[0 : NWARM * 128, :].rearrange(
                    "(wq c) x -> c wq x", c=128
                )
                nc.sync.dma_start(wv4[:, :, 0 : NC4 * 128], s4v)
                nc.sync.dma_start(wv4[:, :, NC4 * 128 : NCI * 128], s3v)
            # zero the tail-chunk region; exp writes only rows [0:QT] there
            nc.gpsimd.memset(expT[:, (NQC - 1) * P : NQC * P], 0.0)

            # broadcast T*w across partitions once: [1,2] -> [128,2]
            with tc.tile_pool(name="wps", bufs=1, space="PSUM") as wps:
                # dummy matmul burst while the PE waits on the first DMAs:
                # ~4us of activity flips the HAM clock-gate to 2.4 GHz so
                # the prep matmuls don't run at the cold 1.2 GHz rate
                warm_ps = wps.tile([128, 128], F32, tag="warmup")
                for i in range(28):
                    mm = nc.tensor.matmul(
                        warm_ps[:], e0[:], e0[:],
                        start=(i == 0), stop=(i == 27),
                        skip_group_check=True,
                    )
                    if i > 0:
                        mm.ins.ldweights = False
                w_ps = wps.tile([128, 2], F32)
                nc.tensor.matmul(w_ps[:], ones_row[:], w_sb[:])
                nc.scalar.copy(w_col[:], w_ps[:])

            # ---------------- prep: query-side normalization ----------------
            # 3 phases so the PE's prep work is not serialized behind the
            # DVE scaling cascade: (1) squares + sum-of-square matmul rows,
            # (2) broadcast + Newton rsqrt, (3) in-place scaling.
            for a, b in ((0, 2), (2, 8), (8, NC4)):
                nc.sync.dma_start(
                    qns[:, a * P : b * P], q4t[:, a * P : b * P]
                )
            for a, b in ((0, 2), (2, NC3)):
                nc.sync.dma_start(
                    qns[:, (NC4 + a) * P : (NC4 + b) * P],
                    q3t[:, a * P : b * P],
                )
            nc.sync.dma_start(fqn[:], fqd[:])
            with ExitStack() as pctx:
                sqpool = pctx.enter_context(tc.tile_pool(name="prepsq", bufs=2))
                mini = pctx.enter_context(tc.tile_pool(name="prepmini", bufs=2))
                pps = pctx.enter_context(
                    tc.tile_pool(name="prepps", bufs=1, space="PSUM")
                )

                qlayers = [(0, NC4, 0), (NC4, NC3, 1)]
                nw_all = {}
                # per-layer pipeline: squares (alternating ACT/DVE) ->
                # sum-of-squares matmuls -> broadcast -> Newton rsqrt ->
                # bf16 w*ninv factor -> in-place scaling.  Layer-4 finishes
                # first so the qc=0 logits matmuls can start while layer-3
                # is still being normalized.
                for ci0, n_ci, wi in qlayers:
                    y0 = float((n_ci * 128) ** -0.5)
                    ss = [
                        pps.tile(
                            [128, PB], F32, tag=f"ss{wi}{pb}", name=f"ss{wi}{pb}"
                        )
                        for pb in range(2)
                    ]
                    groups = [2, 2] + [4] * ((n_ci - 4) // 4)
                    g0 = 0
                    for gi, g in enumerate(groups):
                        sqq = sqpool.tile([128, 4 * P], BF16, tag="sqq")
                        # split each group's squares across ACT and DVE so
                        # the sum-of-squares matmuls are never starved
                        h = g // 2
                        lo = (ci0 + g0) * P
                        nc.scalar.square(
                            sqq[:, 0 : h * P], qns[:, lo : lo + h * P]
                        )
                        nc.vector.tensor_mul(
                            sqq[:, h * P : g * P],
                            qns[:, lo + h * P : lo + g * P],
                            qns[:, lo + h * P : lo + g * P],
                        )
                        for k in range(g):
                            ci = g0 + k
                            for pb in range(2):
                                mm = nc.tensor.matmul(
                                    ss[pb][:],
                                    e0[:],
                                    sqq[:, k * P + pb * PB : k * P + pb * PB + PB],
                                    start=(ci == 0),
                                    stop=(ci == n_ci - 1),
                                )
                                if ci > 0 or pb > 0:
                                    mm.ins.ldweights = False
                        g0 += g
                    nws = []
                    for pb in range(2):
                        u = mini.tile([1, PB], BF16, tag="u")
                        nc.scalar.copy(u[:], ss[pb][0:1, :])
                        bc = pps.tile([128, PB], F32, tag="bc", name=f"bc{wi}{pb}")
                        nc.tensor.matmul(bc[:], ones_row_bf[:], u[:])
                        y1 = mini.tile([128, PB], F32, tag="y1")
                        nc.vector.tensor_scalar(
                            out=y1[:],
                            in0=bc[:],
                            scalar1=-0.5 * y0 * y0 * y0,
                            scalar2=1.5 * y0,
                            op0=MUL,
                            op1=ADD,
                        )
                        t = mini.tile([128, PB], F32, tag="t")
                        nc.vector.tensor_mul(t[:], y1[:], y1[:])
                        nc.vector.tensor_mul(t[:], t[:], bc[:])
                        nc.vector.tensor_scalar(
                            out=t[:], in0=t[:], scalar1=-0.5, scalar2=1.5,
                            op0=MUL, op1=ADD,
                        )
                        ninv = mini.tile([128, PB], F32, tag="ninv")
                        nc.vector.tensor_mul(ninv[:], t[:], y1[:])
                        if pb == 0:
                            nw = nwpool.tile([128, P], BF16, tag=f"nw{wi}")
                            nws.append(nw)
                        nc.vector.scalar_tensor_tensor(
                            nws[0][:, pb * PB : (pb + 1) * PB],
                            ninv[:], w_col[:, wi : wi + 1], zeros_pb[:],
                            MUL, ADD,
                        )
                    nw_all[wi] = nws[0]
                    if wi == 0:
                        # warm-chunk DMAs emitted here: late enough that the
                        # first squares' DMA waits don't cover them, early
                        # enough to be resident before the logits start
                        load_warm()
                # the in-place scaling muls run LAST, in ci order, so the
                # layer-3 norm chain is not stuck behind them on the DVE
                for ci in range(NCI):
                    wi = 0 if ci < NC4 else 1
                    sl = slice(ci * P, (ci + 1) * P)
                    nc.vector.tensor_mul(qns[:, sl], qns[:, sl], nw_all[wi][:])

            # ------------- main: stream support chunks, logits, exp -------------
            with ExitStack() as mctx:
                scpool = mctx.enter_context(tc.tile_pool(name="sc", bufs=3))
                sqpool = mctx.enter_context(tc.tile_pool(name="msq", bufs=2))
                fscpool = mctx.enter_context(tc.tile_pool(name="mfsc", bufs=2))
                invpool = mctx.enter_context(tc.tile_pool(name="minv", bufs=2))
                cmbpool = mctx.enter_context(tc.tile_pool(name="mcmb", bufs=2))
                lps = mctx.enter_context(
                    tc.tile_pool(name="logits", bufs=1, space="PSUM")
                )
                nps = mctx.enter_context(
                    tc.tile_pool(name="normps", bufs=2, space="PSUM")
                )

                for qc in range(NQC):
                    qn = 128 if qc < NQC - 1 else QT
                    r0 = qc * 128
                    if qc < NWARM:
                        sc = warm_sc[:, qc * NCI * 128 : (qc + 1) * NCI * 128]
                    else:
                        sc = scpool.tile([128, NCI * 128], BF16, tag="sc")
                        nc.sync.dma_start(
                            sc[:, 0 : NC4 * 128], s4t[r0 : r0 + 128, :]
                        )
                        nc.sync.dma_start(
                            sc[:, NC4 * 128 : NCI * 128], s3t[r0 : r0 + 128, :]
                        )

                    # support sum-of-squares -> [q,1] norm columns via
                    # pairwise ci folds (squares + first folds on GpSimd)
                    sq = sqpool.tile([128, NCI * 128], BF16, tag="sq")
                    if qc < 2:
                        # DVE is busy with the prep scaling muls here
                        nc.scalar.square(sq[:], sc[:])
                    else:
                        nc.vector.tensor_mul(sq[:], sc[:], sc[:])
                    fsc = fscpool.tile([128, 1536], F32, tag="fsc")
                    nc.vector.tensor_add(
                        fsc[:, 0:1024], sq[:, 0:1024], sq[:, 1024:2048]
                    )
                    nc.vector.tensor_add(
                        fsc[:, 1024:1536], sq[:, 2048:2560], sq[:, 2560:3072]
                    )
                    for lo, n in ((0, 512), (0, 256), (0, 128), (1024, 256), (1024, 128)):
                        nc.vector.tensor_add(
                            fsc[:, lo : lo + n],
                            fsc[:, lo : lo + n],
                            fsc[:, lo + n : lo + 2 * n],
                        )
                    ncps = nps.tile([128, 2], F32, tag="ncp", name=f"ncp{qc}")

                    # logits: D4/D3 accumulate separately; lhsT is the raw
                    # support chunk, reused across both p-blocks
                    psD = [
                        [
                            lps.tile(
                                [128, PB], F32, tag=f"D{ln}{pb}",
                                name=f"D{ln}{pb}", bufs=(2 if ln == 0 else 1),
                            )
                            for pb in range(2)
                        ]
                        for ln in range(2)
                    ]
                    for ci in range(NCI):
                        ln = 0 if ci < NC4 else 1
                        lhsT = sc[:, ci * 128 : (ci + 1) * 128]
                        for pb in range(2):
                            mm = nc.tensor.matmul(
                                psD[ln][pb][:],
                                lhsT,
                                qns[:, ci * P + pb * PB : ci * P + pb * PB + PB],
                                start=(ci == 0 or ci == NC4),
                                stop=(ci == NC4 - 1 or ci == NCI - 1),
                            )
                            if pb == 1:
                                # second matmul of the pair reuses the
                                # weights loaded by the first
                                mm.ins.ldweights = False
                        if ci == 20:
                            # norm-column matmuls tucked behind the first
                            # logits pairs so they never gate the PE; bf16
                            # casts avoid the fp32 double-pumped matmul
                            fbf = invpool.tile([128, 256], BF16, tag="fbf")
                            nc.scalar.copy(fbf[:, 0:128], fsc[:, 0:128])
                            nc.scalar.copy(fbf[:, 128:256], fsc[:, 1024:1152])
                            for ln2 in range(2):
                                nc.tensor.matmul(
                                    ncps[:, ln2 : ln2 + 1],
                                    fbf[:, ln2 * 128 : (ln2 + 1) * 128],
                                    ones_col[:],
                                    skip_group_check=True,
                                )
                    # inv = (sum sq)^(-1/2): Newton rsqrt from constant seeds
                    inv1 = invpool.tile([128, 2], F32, tag="i1")
                    for ln, c in ((0, C4), (1, C3)):
                        y0 = float(c**-0.5)
                        nc.vector.tensor_scalar(
                            out=inv1[:, ln : ln + 1],
                            in0=ncps[:, ln : ln + 1],
                            scalar1=-0.5 * y0 * y0 * y0,
                            scalar2=1.5 * y0,
                            op0=MUL,
                            op1=ADD,
                        )
                    t2 = invpool.tile([128, 2], F32, tag="t2")
                    nc.vector.tensor_mul(t2[:], inv1[:], inv1[:])
                    nc.vector.tensor_mul(t2[:], t2[:], ncps[:])
                    nc.vector.tensor_scalar(
                        out=t2[:], in0=t2[:], scalar1=-0.5, scalar2=1.5,
                        op0=MUL, op1=ADD,
                    )
                    inv = invpool.tile([128, 2], F32, tag="inv")
                    nc.vector.tensor_mul(inv[:], t2[:], inv1[:])
                    for pb in range(2):
                        tmp = cmbpool.tile([128, PB], F32, tag="tmp")
                        nc.vector.scalar_tensor_tensor(
                            tmp[:], psD[1][pb][:], inv[:, 1:2], zeros_pb[:],
                            MUL, ADD,
                        )
                        cmb = cmbpool.tile([128, PB], F32, tag="cmb")
                        nc.vector.scalar_tensor_tensor(
                            cmb[:], psD[0][pb][:], inv[:, 0:1], tmp[:], MUL, ADD
                        )
                        esl = expT[
                            0:qn, qc * P + pb * PB : qc * P + pb * PB + PB
                        ]
                        nc.scalar.activation(esl, cmb[0:qn, :], AF.Exp)

            warm_ctx.close()

            # ---------------- phase B: attention-weighted values ----------------
            with ExitStack() as bctx:
                vpool = bctx.enter_context(tc.tile_pool(name="vtp", bufs=1))
                bps = bctx.enter_context(
                    tc.tile_pool(name="bps", bufs=1, space="PSUM")
                )
                bsq = bctx.enter_context(tc.tile_pool(name="bsq", bufs=2))
                bmini = bctx.enter_context(tc.tile_pool(name="bmini", bufs=1))
                batt = bctx.enter_context(tc.tile_pool(name="batt", bufs=1))
                bout = bctx.enter_context(tc.tile_pool(name="bout", bufs=2))

                # f_q channel-normalization (moved here: only needed by
                # the epilogue; hides under the Y matmuls)
                fsq = bsq.tile([128, NCV * P], BF16, tag="fsq", bufs=1)
                nc.vector.tensor_mul(fsq[:], fqn[:], fqn[:])
                fss = [
                    bps.tile([128, PB], F32, tag=f"ssy{pb}", name=f"fss{pb}")
                    for pb in range(2)
                ]
                for ci in range(NCV):
                    for pb in range(2):
                        mm = nc.tensor.matmul(
                            fss[pb][:],
                            e0[:],
                            fsq[:, ci * P + pb * PB : ci * P + pb * PB + PB],
                            start=(ci == 0),
                            stop=(ci == NCV - 1),
                        )
                        if ci > 0 or pb > 0:
                            mm.ins.ldweights = False
                y0f = float(CV**-0.5)
                for pb in range(2):
                    u = bmini.tile([1, PB], BF16, tag=f"uf{pb}")
                    nc.scalar.copy(u[:], fss[pb][0:1, :])
                    bc = bps.tile([128, PB], F32, tag="bcscr", name=f"fbc{pb}")
                    nc.tensor.matmul(bc[:], ones_row_bf[:], u[:])
                    y1 = bmini.tile([128, PB], F32, tag="y1f")
                    nc.vector.tensor_scalar(
                        out=y1[:], in0=bc[:], scalar1=-0.5 * y0f * y0f * y0f,
                        scalar2=1.5 * y0f, op0=MUL, op1=ADD,
                    )
                    t = bmini.tile([128, PB], F32, tag="tf")
                    nc.vector.tensor_mul(t[:], y1[:], y1[:])
                    nc.vector.tensor_mul(t[:], t[:], bc[:])
                    nc.vector.tensor_scalar(
                        out=t[:], in0=t[:], scalar1=-0.5, scalar2=1.5,
                        op0=MUL, op1=ADD,
                    )
                    ninv = bmini.tile([128, PB], F32, tag=f"ninvf{pb}")
                    nc.vector.tensor_mul(ninv[:], t[:], y1[:])
                    for ci in range(NCV):
                        sl = slice(ci * P + pb * PB, ci * P + pb * PB + PB)
                        nc.vector.tensor_mul(fqn[:, sl], fqn[:, sl], ninv[:])

                # stream f_s.T directly as bf16 (pad rows are zero on host)
                vt_all = vpool.tile([128, NQC * CV], BF16)
                vtv = vt_all[:].rearrange("q (qc v) -> q qc v", qc=NQC)
                srcv = vtd[:].rearrange("(qc q) v -> q qc v", q=128)
                for qc0 in range(0, NQC, 8):
                    g = min(8, NQC - qc0)
                    nc.sync.dma_start(
                        vtv[:, qc0 : qc0 + g, :], srcv[:, qc0 : qc0 + g, :]
                    )

                # softmax denominators + 1/denominator broadcast; the psum
                # pool is scoped so its banks free up for the Y matmuls
                bcd_sb, bcd_raw = [], []
                with tc.tile_pool(name="dnps", bufs=1, space="PSUM") as dnps:
                    dns = [
                        dnps.tile(
                            [128, PB], F32, tag=f"dn{pb}", name=f"dn{pb}"
                        )
                        for pb in range(2)
                    ]
                    for qc in range(NQC):
                        for pb in range(2):
                            mm = nc.tensor.matmul(
                                dns[pb][:],
                                e0[:],
                                expT[:, qc * P + pb * PB : qc * P + pb * PB + PB],
                                start=(qc == 0),
                                stop=(qc == NQC - 1),
                            )
                            if qc > 0 or pb > 0:
                                mm.ins.ldweights = False
                    for pb in range(2):
                        u = bmini.tile([1, PB], F32, tag=f"ud{pb}")
                        nc.scalar.copy(u[:], dns[pb][0:1, :])
                        bcp = bps.tile([128, PB], F32, tag="bcscr", name=f"bd{pb}")
                        nc.tensor.matmul(bcp[:], ones_row[:], u[:])
                        raw = bmini.tile([128, PB], F32, tag=f"dnraw{pb}")
                        nc.scalar.copy(raw[:], bcp[:])
                        inv = bmini.tile([128, PB], F32, tag=f"dninv{pb}")
                        act_table(inv[:], bcp[:], AF.Reciprocal)
                        bcd_sb.append(inv)
                        bcd_raw.append(raw)

                yps = bctx.enter_context(
                    tc.tile_pool(name="yps", bufs=2, space="PSUM")
                )
                ssy = [
                    bps.tile([128, PB], F32, tag=f"ssy{pb}", name=f"ssy{pb}")
                    for pb in range(2)
                ]
                att_sb = {}
                for cb in range(NCV):
                    ys = [
                        yps.tile([128, PB], F32, tag=f"y{pb}", name=f"y{cb}_{pb}")
                        for pb in range(2)
                    ]
                    for qc in range(NQC):
                        lhsT = vt_all[:, qc * CV + cb * 128 : qc * CV + (cb + 1) * 128]
                        for pb in range(2):
                            mm = nc.tensor.matmul(
                                ys[pb][:],
                                lhsT,
                                expT[:, qc * P + pb * PB : qc * P + pb * PB + PB],
                                start=(qc == 0),
                                stop=(qc == NQC - 1),
                            )
                            if pb == 1:
                                mm.ins.ldweights = False
                    for pb in range(2):
                        att = batt.tile(
                            [128, PB], F32, tag=f"att{cb}_{pb}", name=f"att{cb}_{pb}"
                        )
                        nc.vector.tensor_mul(att[:], ys[pb][:], bcd_sb[pb][:])
                        att_sb[(cb, pb)] = att
                        nc.sync.dma_start(
                            att_o[cb * 128 : (cb + 1) * 128, pb * PB : (pb + 1) * PB],
                            att[:],
                        )
                        sqy = bsq.tile([128, PB], BF16, tag="sqy")
                        nc.scalar.square(sqy[:], ys[pb][:])
                        mm = nc.tensor.matmul(
                            ssy[pb][:],
                            e0[:],
                            sqy[:],
                            start=(cb == 0),
                            stop=(cb == NCV - 1),
                        )
                        if pb == 1:
                            mm.ins.ldweights = False

                for pb in range(2):
                    u = bmini.tile([1, PB], F32, tag=f"us{pb}")
                    nc.scalar.copy(u[:], ssy[pb][0:1, :])
                    bcp = bps.tile([128, PB], F32, tag="bcscr", name=f"bs{pb}")
                    nc.tensor.matmul(bcp[:], ones_row[:], u[:])
                    # rsqrt(ssy/ATT_WT^2) = 0.3/||Y|| in one ACT op
                    sinv = bmini.tile([128, PB], F32, tag=f"sinv{pb}")
                    act_table(
                        sinv[:], bcp[:], AF.Rsqrt,
                        scale=float(1.0 / (ATT_WT * ATT_WT)),
                    )
                    # fq = fqn + att * (denom * 0.3/||Y||)
                    s2 = bmini.tile([128, PB], F32, tag=f"s2{pb}")
                    nc.vector.tensor_mul(s2[:], bcd_raw[pb][:], sinv[:])
                    for cb in range(NCV):
                        t = bout.tile([128, PB], F32, tag=f"t{pb}")
                        nc.vector.tensor_mul(t[:], att_sb[(cb, pb)][:], s2[:])
                        f_sb = bout.tile([128, PB], F32, tag=f"f{pb}")
                        nc.vector.tensor_add(
                            f_sb[:],
                            t[:],
                            fqn[:, cb * P + pb * PB : cb * P + pb * PB + PB],
                        )
                        nc.sync.dma_start(
                            fq_o[cb * 128 : (cb + 1) * 128, pb * PB : (pb + 1) * PB],
                            f_sb[:],
                        )
    _split_sync_waits(nc)
    return nc


def _tile_support(x, n_ci):
    """[C, HW] f32 -> [HWP, C] bf16 with s[qc*128+c, ci*128+q] layout."""
    a = np.asarray(x, np.float32).reshape(n_ci, 128, HW)
    a = np.concatenate(
        [a, np.zeros((n_ci, 128, HWP - HW), np.float32)], axis=2
    )
    a = a.reshape(n_ci, 128, NQC, 128).transpose(2, 1, 0, 3).reshape(HWP, n_ci * 128)
    return np.ascontiguousarray(a.astype(NP_BF16))


def _tile_query(x, n_ci, dtype):
    """[C, P] -> [128, n_ci*P] with q[c, ci*P + p] layout."""
    a = np.asarray(x, np.float32).reshape(n_ci, 128, P).transpose(1, 0, 2)
    return np.ascontiguousarray(a.reshape(128, n_ci * P).astype(dtype))


def make_in_maps(fq_l3, fs_l3, fq_l4, fs_l4, f_q, f_s, w_red):
    wvec = np.asarray(
        [[TEMP * float(w_red[0]), TEMP * float(w_red[1])]], dtype=np.float32
    )
    per_batch = []
    for b in range(B):
        s4 = _tile_support(np.asarray(fs_l4, np.float32)[b].reshape(C4, HW), NC4)
        s3 = _tile_support(np.asarray(fs_l3, np.float32)[b].reshape(C3, HW), NC3)
        vt = np.zeros((HWP, CV), np.float32)
        vt[:HW] = np.asarray(f_s, np.float32)[b].reshape(CV, HW).T
        vt = np.ascontiguousarray(vt.astype(NP_BF16))
        per_batch.append((s4, s3, vt))
    q4f = np.asarray(fq_l4, np.float32).reshape(B, C4, HW)
    q3f = np.asarray(fq_l3, np.float32).reshape(B, C3, HW)
    fqf = np.asarray(f_q, np.float32).reshape(B, CV, HW)
    in_maps = []
    for k in range(NCORES):
        b, j = divmod(k, PSH)
        sl = slice(j * P, (j + 1) * P)
        s4, s3, vt = per_batch[b]
        in_maps.append(
            {
                "s4t": s4,
                "s3t": s3,
                "vtd": vt,
                "q4t": _tile_query(q4f[b][:, sl], NC4, NP_BF16),
                "q3t": _tile_query(q3f[b][:, sl], NC3, NP_BF16),
                "fqd": _tile_query(fqf[b][:, sl], NCV, np.float32),
                "wv": wvec,
            }
        )
    return in_maps


def gather_outputs(results):
    att = np.empty((B, CV, HW), np.float32)
    fqo = np.empty((B, CV, HW), np.float32)
    for k in range(NCORES):
        b, j = divmod(k, PSH)
        sl = slice(j * P, (j + 1) * P)
        att[b][:, sl] = results[k]["att_o"]
        fqo[b][:, sl] = results[k]["fq_o"]
    return (
        fqo.reshape(B, CV, H, W),
        att.reshape(B, CV, H, W),
    )


def kernel(fq_l3, fs_l3, fq_l4, fs_l4, f_q, f_s, w_red, trace=False):
    nc = build()
    in_maps = make_in_maps(fq_l3, fs_l3, fq_l4, fs_l4, f_q, f_s, w_red)
    res = run_bass_kernel_spmd(nc, in_maps, core_ids=list(range(NCORES)), trace=trace)
    out = gather_outputs(res.results)
    if trace:
        return out, res
    return out
